# revision 8
# baseline (speedup 1.0000x reference)
"""TRN2 Bass kernel for nn_GCNBlock_77927886618861 (gnn_message_passing).

Reference computation (per batch b, K=5 neighbors, H=8192 positions):
    d = diff_patch.reshape(b,5,192,H)
    x = w1@d + b1; r = relu(wr1@x + br1); r = wr2@r + br2; x = x + 0.1*r
    logits = w3@x + b3; wgt = softmax_k(logits)
    knn = sum_k knn_hr_k * wgt_k;  knn_lr = mean-pool(knn, scale=2)

The 1x1-conv MLP collapses algebraically (softmax over k is invariant to
per-position constants, so all constant terms drop):
    logits'_k = a.d_k + v.relu(G.d_k + g0)
    G = wr1@w1 [64,192], g0 = wr1@b1+br1, a = w3@w1 [192], v = 0.1*w3@wr2 [64]

The a.d_k term rides along as rows 64/65 of an extended Gt = [G; a; -a]
with zero bias there: relu(a.d) - relu(-a.d) == a.d exactly (the v-matmul
column carries +1/-1 for those rows), avoiding any magnitude shift that
would inflate float32r's relative rounding into logit error.

Sharding: one batch (of 8) per NeuronCore. Matmuls in float32r (1 cyc/col
warm). Softmax normalization via reciprocal_approx_fast on an unnormalized
sum that rides as row 64 of the k=4 broadcast matmul; weights stay
unnormalized until a single fused (hr-weighted-sum * 1/s) DVE multiply.
The k-sum of hr*wgt uses PE fold matmuls (cross-partition adds are illegal
on DVE/ACT/GPSIMD).

DMA strategy: HWDGE descriptor-gen is scarce (~15ns/descriptor/ring), so
diff_patch is host-rearranged for 10KB contiguous runs and loads are spread
across all three DGE paths (gpsimd SWDGE, sync + scalar HWDGE rings);
outputs are written per 4-tile group from SBUF-resident accumulators.
"""
import sys

import numpy as np

sys.path.insert(0, "/opt/trn_rl_repo")

import concourse.bass as bass  # noqa: E402
import concourse.bacc as bacc  # noqa: E402
import concourse.tile as tile  # noqa: E402
from concourse import mybir  # noqa: E402
from concourse.bass_utils import run_bass_kernel_spmd  # noqa: E402

F32 = mybir.dt.float32
F32R = mybir.dt.float32r
AF = mybir.ActivationFunctionType
ALU = mybir.AluOpType

B, K, C, H, DN, SCALE = 8, 5, 64, 8192, 192, 2
RES_SCALE = 0.1
NT = 512                 # positions per tile
NTILES = H // NT         # 16
GRP = 4                  # hr tiles loaded / outputs stored per group
HHALF = H // SCALE       # 4096
CE = C + 2               # 66 rows: G plus the [a; -a] pair

_CACHE = {}


def _bcast_row(row, parts):
    """AP reading one [1, N] SBUF row replicated across `parts` partitions."""
    return bass.AP(tensor=row.tensor, offset=row.offset,
                   ap=[row.ap[0], [0, parts], row.ap[1]])


def _build_nc():
    nc = bacc.Bacc("TRN2", target_bir_lowering=False, debug=False)

    # d is host-rearranged to [NTILES, 192, K*NT]: per tile, channel-major
    # rows of K*NT contiguous floats (10KB descriptors).
    d_d = nc.dram_tensor("d", [NTILES, DN, K * NT], F32R, kind="ExternalInput")
    hr_d = nc.dram_tensor("hr", [K * C, H], F32, kind="ExternalInput")
    gt_hi_d = nc.dram_tensor("gt_hi", [128, CE], F32R, kind="ExternalInput")
    gt_lo_d = nc.dram_tensor("gt_lo", [64, CE], F32R, kind="ExternalInput")
    v65_d = nc.dram_tensor("v65", [CE, K, K], F32R, kind="ExternalInput")
    bc01_d = nc.dram_tensor("bc01", [K, 128], F32R, kind="ExternalInput")
    bc23_d = nc.dram_tensor("bc23", [K, 128], F32R, kind="ExternalInput")
    bc4_d = nc.dram_tensor("bc4", [K, 65], F32R, kind="ExternalInput")
    ones5_d = nc.dram_tensor("ones5", [K, 1], F32R, kind="ExternalInput")
    fold2_d = nc.dram_tensor("fold2", [128, C], F32R, kind="ExternalInput")
    i64_d = nc.dram_tensor("i64", [C, C], F32R, kind="ExternalInput")
    g0p_d = nc.dram_tensor("g0p", [CE, 1], F32, kind="ExternalInput")

    knn_d = nc.dram_tensor("knn", [C, H], F32, kind="ExternalOutput")
    lr_d = nc.dram_tensor("knn_lr", [C, HHALF], F32, kind="ExternalOutput")

    with tile.TileContext(nc) as tc:
        with (
            tc.tile_pool(name="wts", bufs=1) as wts,
            tc.tile_pool(name="dbuf", bufs=3) as dbuf,
            tc.tile_pool(name="hrbuf", bufs=2) as hrbuf,
            tc.tile_pool(name="hbuf", bufs=4) as hbuf,
            tc.tile_pool(name="smx", bufs=2) as smx,
            tc.tile_pool(name="prod", bufs=3) as prod,
            tc.tile_pool(name="big", bufs=1) as bigp,
            tc.tile_pool(name="lrp", bufs=2) as lrp,
            tc.tile_pool(name="pheb", bufs=3, space="PSUM") as pheb,
            tc.tile_pool(name="pl", bufs=2, space="PSUM") as plp,
            tc.tile_pool(name="pknn", bufs=2, space="PSUM") as pknnp,
            tc.tile_pool(name="pss", bufs=1, space="PSUM") as pssp,
        ):
            # ---- constants (one-time) ----
            gt_hi = wts.tile([128, CE], F32R)
            gt_lo = wts.tile([64, CE], F32R)
            v65 = wts.tile([CE, K, K], F32R)
            bc01 = wts.tile([K, 128], F32R)
            bc23 = wts.tile([K, 128], F32R)
            bc4 = wts.tile([K, 65], F32R)
            ones5 = wts.tile([K, 1], F32R)
            fold2 = wts.tile([128, C], F32R)
            i64 = wts.tile([C, C], F32R)
            g0p = wts.tile([CE, 1], F32)
            for t, dd in (
                (gt_hi, gt_hi_d), (gt_lo, gt_lo_d), (v65, v65_d),
                (bc01, bc01_d), (bc23, bc23_d),
                (bc4, bc4_d), (ones5, ones5_d),
                (fold2, fold2_d), (i64, i64_d), (g0p, g0p_d),
            ):
                nc.sync.dma_start(out=t, in_=dd[:])

            knn_sb = bigp.tile([C, H], F32)    # full-batch knn for mean-pool
            lr_sb = bigp.tile([C, HHALF], F32)

            hr_g = [None, None, None]
            for j in range(NTILES):
                n0 = j * NT
                jg = j % GRP
                # ---- loads ----
                if jg == 0:
                    gg = j * NT
                    hr01 = hrbuf.tile([128, GRP, NT], F32, tag="hr01")
                    hr23 = hrbuf.tile([128, GRP, NT], F32, tag="hr23")
                    hr4 = hrbuf.tile([64, GRP, NT], F32, tag="hr4")
                    nc.sync.dma_start(
                        out=hr01, in_=hr_d[0:128, gg : gg + GRP * NT])
                    nc.sync.dma_start(
                        out=hr23, in_=hr_d[128:256, gg : gg + GRP * NT])
                    nc.sync.dma_start(
                        out=hr4, in_=hr_d[256:320, gg : gg + GRP * NT])
                    hr_g = [hr01, hr23, hr4]
                dhi = dbuf.tile([128, K, NT], F32R, tag="dhi")
                dlo = dbuf.tile([64, K, NT], F32R, tag="dlo")
                nc.gpsimd.dma_start(
                    out=dhi[:].rearrange("c k n -> c (k n)"),
                    in_=d_d[j, 0:128, :])
                nc.scalar.dma_start(
                    out=dlo[:].rearrange("c k n -> c (k n)"),
                    in_=d_d[j, 128:192, :])

                # ---- per-neighbor MLP + logits (pairwise for PE density) ----
                pl = plp.tile([K, NT], F32)
                for ks in ((0, 1), (2, 3), (4,)):
                    phs = []
                    for k in ks:
                        ph = pheb.tile([CE, NT], F32, tag="psum_big",
                                       name=f"ph{k}")
                        nc.tensor.matmul(ph[:], gt_hi[:], dhi[:, k, :],
                                         start=True, stop=False)
                        nc.tensor.matmul(ph[:], gt_lo[:], dlo[:, k, :],
                                         start=False, stop=True)
                        phs.append(ph)
                    hks = []
                    for k, ph in zip(ks, phs):
                        # h = relu([G;a;-a].d_k + [g0;0;0])
                        h_k = hbuf.tile([CE, NT], F32R, tag="h", name=f"h{k}")
                        nc.scalar.activation(h_k[:], ph[:], AF.Relu,
                                             bias=g0p[:])
                        hks.append(h_k)
                    for k, h_k in zip(ks, hks):
                        # logits row k += v.h_k + relu(a.d_k) - relu(-a.d_k)
                        nc.tensor.matmul(pl[:], v65[:, k, :], h_k[:],
                                         start=(k == 0), stop=(k == K - 1),
                                         skip_group_check=True)

                # ---- softmax numerator + channel broadcast (PE matmuls) ----
                e5 = smx.tile([K, NT], F32R, tag="e5")
                nc.scalar.activation(e5[:], pl[:], AF.Exp)
                eb01 = pheb.tile([128, NT], F32, tag="psum_big")
                eb23 = pheb.tile([128, NT], F32, tag="psum_big")
                eb4s = pheb.tile([65, NT], F32, tag="psum_big")
                nc.tensor.matmul(eb01[:], bc01[:], e5[:], start=True, stop=True)
                nc.tensor.matmul(eb23[:], bc23[:], e5[:], start=True, stop=True)
                # rows 0..63 = e4 broadcast; row 64 = sum_k e_k
                nc.tensor.matmul(eb4s[:], bc4[:], e5[:], start=True, stop=True)
                ps_s = pssp.tile([1, NT], F32, tag="ps_s")
                nc.tensor.matmul(ps_s[:], ones5[:], e5[:], start=True, stop=True)
                r_sb = smx.tile([1, NT], F32, tag="r_sb")
                nc.vector.reciprocal_approx_fast(
                    out=r_sb[:], in_=ps_s[:])
                rbc64 = smx.tile([C, NT], F32, tag="rbc64")
                nc.scalar.dma_start(out=rbc64,
                                    in_=_bcast_row(r_sb[0:1, :], C))

                # ---- weighted products + PE fold over k ----
                p01 = prod.tile([128, NT], F32R, tag="p01")
                p23 = prod.tile([128, NT], F32R, tag="p23")
                p4 = prod.tile([64, NT], F32R, tag="p4")
                nc.vector.tensor_mul(p01[:], hr_g[0][:, jg, :], eb01[:])
                nc.vector.tensor_mul(p23[:], hr_g[1][:, jg, :], eb23[:])
                nc.vector.tensor_mul(p4[:], hr_g[2][:, jg, :], eb4s[0:64, :])
                pk = pknnp.tile([C, NT], F32)
                nc.tensor.matmul(pk[:], fold2[:], p01[:], start=True, stop=False)
                nc.tensor.matmul(pk[:], fold2[:], p23[:], start=False, stop=False)
                nc.tensor.matmul(pk[:], i64[:], p4[:], start=False, stop=True)
                # knn = (sum_k hr_k * e_k) / sum_k e_k
                nc.vector.tensor_mul(knn_sb[:, n0 : n0 + NT], pk[:], rbc64[:])

                # ---- interleaved mean-pool once the paired tile exists ----
                if j >= HHALF // NT:
                    m0 = n0 - HHALF
                    t = lrp.tile([C, NT], F32, tag="lrt")
                    nc.vector.tensor_add(
                        t[:], knn_sb[:, m0 : m0 + NT],
                        knn_sb[:, n0 : n0 + NT],
                    )
                    nc.scalar.activation(lr_sb[:, m0 : m0 + NT], t[:],
                                         AF.Copy, scale=0.5)

                # ---- grouped output stores ----
                if jg == GRP - 1:
                    gg = (j - GRP + 1) * NT
                    nc.sync.dma_start(out=knn_d[:, gg : gg + GRP * NT],
                                      in_=knn_sb[:, gg : gg + GRP * NT])
                    if j >= HHALF // NT:
                        mg = gg - HHALF
                        nc.sync.dma_start(out=lr_d[:, mg : mg + GRP * NT],
                                          in_=lr_sb[:, mg : mg + GRP * NT])

    nc.compile()
    return nc


def _consts(w1, b1, wr1, br1, wr2, br2, w3, b3):
    w1, b1, wr1, br1, wr2, br2, w3, b3 = (
        np.asarray(t, np.float64) for t in (w1, b1, wr1, br1, wr2, br2, w3, b3)
    )
    G = wr1 @ w1                      # [64, 192]
    g0 = wr1 @ b1 + br1               # [64]
    a = (w3 @ w1)[0]                  # [192]
    v = RES_SCALE * (w3 @ wr2)[0]     # [64]

    Ge = np.concatenate([G, a[None, :], -a[None, :]], axis=0)  # [66, 192]
    gte = Ge.T.astype(np.float32).copy()            # [192, 66]
    v65 = np.zeros((CE, K, K), np.float32)
    for k in range(K):
        v65[0:C, k, k] = v
        v65[C, k, k] = 1.0
        v65[C + 1, k, k] = -1.0
    bc01 = np.zeros((K, 128), np.float32)
    bc01[0, 0:64] = 1.0
    bc01[1, 64:128] = 1.0
    bc23 = np.zeros((K, 128), np.float32)
    bc23[2, 0:64] = 1.0
    bc23[3, 64:128] = 1.0
    bc4 = np.zeros((K, 65), np.float32)
    bc4[4, 0:64] = 1.0
    bc4[:, 64] = 1.0                  # row 64 of eb4s = sum_k e_k
    fold2 = np.zeros((128, C), np.float32)
    for r in range(128):
        fold2[r, r % 64] = 1.0
    g0p = np.concatenate([g0, [0.0, 0.0]]).astype(np.float32).reshape(CE, 1)
    return {
        "gt_hi": np.ascontiguousarray(gte[0:128]),
        "gt_lo": np.ascontiguousarray(gte[128:192]),
        "v65": v65,
        "bc01": bc01,
        "bc23": bc23,
        "bc4": bc4,
        "ones5": np.ones((K, 1), np.float32),
        "fold2": fold2,
        "i64": np.eye(C, dtype=np.float32),
        "g0p": g0p,
    }


def kernel(knn_hr, diff_patch, w1, b1, wr1, br1, wr2, br2, w3, b3, **kw):
    knn_hr = np.ascontiguousarray(np.asarray(knn_hr, np.float32))
    diff_patch = np.asarray(diff_patch, np.float32)

    # [B, 5*192, H] -> [B, NTILES, 192, K*NT]: per tile, channel rows hold
    # all 5 neighbors' NT positions contiguously.
    d_re = np.ascontiguousarray(
        diff_patch.reshape(B, K, DN, NTILES, NT).transpose(0, 3, 2, 1, 4)
    ).reshape(B, NTILES, DN, K * NT)

    if "nc" not in _CACHE:
        _CACHE["nc"] = _build_nc()
    nc = _CACHE["nc"]

    consts = _consts(w1, b1, wr1, br1, wr2, br2, w3, b3)
    in_maps = []
    for b in range(B):
        m = dict(consts)
        m["d"] = d_re[b]
        m["hr"] = knn_hr[b]
        in_maps.append(m)

    res = run_bass_kernel_spmd(nc, in_maps, core_ids=list(range(B)))
    knn = np.stack([res.results[b]["knn"] for b in range(B)])
    knn_lr = np.stack([res.results[b]["knn_lr"] for b in range(B)])
    return knn_lr, knn
